# revision 47
# baseline (speedup 1.0000x reference)
"""DinoSwav attention-crop kernel for Trainium2 (Bass/Tile), 8-core data parallel.

Per sample:
  1. CLS-attention map (mean over heads) -> 14x14, bilinear-upsampled to 224x224
  2. threshold mask (> 0.6*max), row/col any -> bbox with 10% margin
  3. crop + bilinear resize back to 224x224

Everything runs on device. The crop-resize is expressed as two fp16 matmuls per
channel with data-dependent interpolation matrices Wy/Wx built on-device from
the bbox via a rank-3 outer-difference matmul (int+frac coordinate split keeps
fp16 exact) + min(|d|,1)-1 (negated weights; the sign cancels across the two
matmul stages).
"""

import numpy as np
from contextlib import ExitStack

import concourse.bass as bass
import concourse.tile as tile
from concourse import mybir
from concourse.bass_utils import run_bass_kernel_spmd

F32 = mybir.dt.float32
F16 = mybir.dt.float16
I32 = mybir.dt.int32
AX = mybir.AxisListType
OP = mybir.AluOpType
ACTF = mybir.ActivationFunctionType

IMG = 224
G = 14          # patch grid
HEADS = 6
PB = 16         # samples per core
NC_CORES = 8
H = 112         # half of IMG (partition tile)
NPAD = 256      # padded matmul free dim; two blocks pack into one PSUM bank
BIG = 1 << 20   # sentinel for masked argmin/argmax; exact in f32


def _upsample_matrix():
    """U[x, j]: bilinear 14 -> 224 weights (half-pixel centers). Exact in fp16."""
    s = (np.arange(IMG, dtype=np.float64) + 0.5) * G / IMG - 0.5
    s = np.clip(s, 0.0, G - 1)
    U = np.maximum(0.0, 1.0 - np.abs(s[:, None] - np.arange(G)[None, :]))
    return U.astype(np.float32)  # (224, 14)


def _host_consts():
    U = _upsample_matrix()
    c = {}
    u_rep = np.zeros((HEADS * G, NPAD), np.float32)
    u_rep[:, :IMG] = np.tile(U.T, (HEADS, 1))
    c["u_rep"] = u_rep.astype(np.float16)
    for h in range(2):
        ut = np.zeros((G, 128), np.float32)
        ut[:, :H] = U[h * H:(h + 1) * H, :].T
        c[f"u_t_h{h}"] = ut.astype(np.float16)  # (14, 128): M=128 enables FWL
    u_t_full = np.zeros((G, NPAD), np.float32)
    u_t_full[:, :IMG] = U.T
    c["u_t_full"] = u_t_full.astype(np.float16)
    c["identity112"] = np.eye(H, dtype=np.float32)
    ar = np.arange(IMG, dtype=np.int64)
    c["iota_mB"] = np.tile((ar - BIG).astype(np.float32), (PB, 1))   # (16, 224)
    c["iota_pB"] = np.tile((ar + BIG).astype(np.float32), (PB, 1))
    c["iota_half_f"] = np.tile((ar + 0.5).astype(np.float32), (PB, 1))
    c["thr_pad"] = np.tile((np.arange(1, 23) * 65536.0).astype(np.float32), (PB, 1))
    for h in range(2):
        w = np.ones((3, 128), np.float32)    # rows: [gi, gf, -x], x = h*112+p
        w[2, :] = -1000.0
        w[2, :H] = -(h * H + np.arange(H, dtype=np.float32))
        c[f"lhsT_wx{h}"] = w.astype(np.float16)
    for t in range(2):
        w = np.ones((3, 128), np.float32)    # rows: [gi, gf, -y], y = 2p+t
        w[2, :] = -1000.0
        w[2, :H] = -(2.0 * np.arange(H, dtype=np.float32) + t)
        c[f"lhsT_wy{t}"] = w.astype(np.float16)
    r2i = np.zeros((3, PB * NPAD), np.float32)
    r2i[2, :] = 1.0
    c["r2_init"] = r2i.astype(np.float16)
    return c


def _split_multi_waits(nc, max_waits=1):
    """The walrus build in this environment accepts only one sync-wait per
    instruction; hoist extra waits onto same-engine NOPs placed just before."""
    ctr = 0
    for fn in nc.m.functions:
        for blk in fn.blocks:
            lst = blk.instructions
            out = []
            changed = False
            for ins in lst:
                si = ins.sync_info
                if si is not None and len(si.on_wait) > max_waits:
                    waits = list(si.on_wait)
                    hoist, keep = waits[:-max_waits], waits[-max_waits:]
                    for w in hoist:
                        ctr += 1
                        nop = mybir.InstNoOp(
                            name=f"waitsplit-{ctr}",
                            engine=ins.engine,
                            ins=[], outs=[],
                            sync_info=mybir.SyncInfo(on_wait=[w], on_update=[]),
                        )
                        out.append(nop)
                    si.on_wait = keep
                    changed = True
                out.append(ins)
            if changed:
                blk.instructions = out


def _build_program(split_waits=True):
    nc = bass.Bass()
    xin = nc.declare_dram_parameter("xin", [PB * 3 * IMG * IMG + 64], F16, isOutput=False)
    attn = nc.declare_dram_parameter("attn", [HEADS * G, PB, G], F16, isOutput=False)
    consts = _host_consts()
    cdecl = {}
    for name, arr in consts.items():
        dt = {np.dtype(np.float32): F32, np.dtype(np.float16): F16,
              np.dtype(np.int32): I32}[arr.dtype]
        cdecl[name] = nc.declare_dram_parameter(name, list(arr.shape), dt, isOutput=False)
    crops_o = nc.declare_dram_parameter("crops", [PB, 3, IMG, IMG], F32, isOutput=True)
    bbox_o = nc.declare_dram_parameter("bboxes", [PB, 4], I32, isOutput=True)
    warm_scr = nc.dram_tensor("warm_scr", [H, 2 * NPAD], F16)
    GS = 8  # bbox group size: lets interp(g) overlap bbox(g+1)

    with tile.TileContext(nc) as tc, ExitStack() as ctx:
        singles = ctx.enter_context(tc.tile_pool(name="singles", bufs=1))
        tpairp = ctx.enter_context(tc.tile_pool(name="tpair", bufs=3))
        wpool = ctx.enter_context(tc.tile_pool(name="wpool", bufs=6))
        wabsp = ctx.enter_context(tc.tile_pool(name="wabs", bufs=3))
        imgp = ctx.enter_context(tc.tile_pool(name="imgp", bufs=6))
        tmpsb = ctx.enter_context(tc.tile_pool(name="tmpsb", bufs=6))
        cropsb = ctx.enter_context(tc.tile_pool(name="cropsb", bufs=6))
        smallp = ctx.enter_context(tc.tile_pool(name="smallp", bufs=1))
        ps_mm = ctx.enter_context(tc.tile_pool(name="ps_mm", bufs=3, space="PSUM"))
        ps_tmp = ctx.enter_context(tc.tile_pool(name="ps_tmp", bufs=3, space="PSUM"))
        ps_crop = ctx.enter_context(tc.tile_pool(name="ps_crop", bufs=2, space="PSUM"))

        # ---- load constants into SBUF, round-robin across engine queues so
        #      the ~650ns-per-DMA issue cost is parallel, not serial ----
        csb = {}
        dma_engs = [nc.sync, nc.gpsimd]
        ei = 0
        for name, arr in consts.items():
            if name == "r2_init":
                continue
            t = singles.tile(list(arr.shape), cdecl[name].dtype, tag=name)
            dma_engs[ei % len(dma_engs)].dma_start(out=t[:], in_=cdecl[name][:])
            ei += 1
            csb[name] = t

        lhsT_a = singles.tile([HEADS * G, PB, G], F16, tag="lhsT_a")
        nc.sync.dma_start(out=lhsT_a[:], in_=attn[:])

        # ---- persistent small tensors ----
        Ry = [singles.tile([H, PB], F32, tag=f"Ry{h}", name=f"Ry{h}") for h in range(2)]
        Cxa = singles.tile([H, 2, PB], F32, tag="Cxa", name="Cxa")
        r2 = {a: singles.tile([3, PB * NPAD], F16, tag=f"r2{a}", name=f"r2{a}")
              for a in ("y", "x")}
        nc.sync.dma_start(out=r2["y"][:], in_=cdecl["r2_init"][:])
        nc.gpsimd.dma_start(out=r2["x"][:], in_=cdecl["r2_init"][:])

        # ---- PE warm-up: ~5us of dense matmuls releases the HAM clock gate ----
        wsrc = singles.tile([128, 2 * NPAD], F16, tag="wsrc")
        nc.vector.memset(wsrc[:], 1.0)
        wps = ps_mm.tile([128, 2 * NPAD], F32, tag="mm", name="wps")
        NWARM = 8
        for i in range(NWARM):
            nc.tensor.matmul(wps[:], wsrc[:, 0:128], wsrc[:],
                             start=(i == 0), stop=(i == NWARM - 1))
        wout = singles.tile([H, 2 * NPAD], F16, tag="wout")
        nc.scalar.copy(wout[:], wps[0:H, :])
        nc.sync.dma_start(out=warm_scr[:], in_=wout[:])

        def st(shape, dtype, tag):
            return smallp.tile(shape, dtype, tag=tag, name=tag)

        biasm05 = st([GS, 1], F32, "biasm05")
        nc.vector.memset(biasm05[:], -0.5)

        def bbox_group(g):
            """T' + amap/amapT matmuls + row/col maxes for samples [g*GS, (g+1)*GS)."""
            for b0 in range(g * GS, (g + 1) * GS, 2):
                tpair = tpairp.tile([G, 2 * NPAD], F16, tag="tpair", name="tpair")
                nc.vector.memset(tpair[:, 2 * IMG:2 * NPAD], 0.0)
                for si in range(2):
                    b = b0 + si
                    tp = ps_mm.tile([G, NPAD], F32, tag="mm", name="tp")
                    nc.tensor.matmul(tp[:, 0:IMG], lhsT_a[:, b, :],
                                     csb["u_rep"][:, 0:IMG])
                    nc.scalar.copy(tpair[:, si * IMG:(si + 1) * IMG], tp[:, 0:IMG])
                # upsample-x on partitions: X profile for both samples at once.
                # Along the reduced (y) axis the map is piecewise linear between
                # grid nodes at pixels 16i+7.5, so per-segment pixel maxima are
                # at columns {16i+7, 16i+8}: reducing over those 28 is exact.
                ysel = tpair[:, 0:2 * IMG].rearrange(
                    "p (s a c) -> p s a c", s=2, c=16)[:, :, :, 7:9]
                for h in range(2):
                    am = ps_mm.tile([128, 2 * 28], F32, tag="mm", name="am")
                    nc.tensor.matmul(am[:], csb[f"u_t_h{h}"][:], ysel)
                    nc.vector.tensor_reduce(
                        Ry[h][:, b0:b0 + 2],
                        am[0:H, :].rearrange("p (s c) -> p s c", s=2),
                        axis=AX.X, op=OP.max)
                # upsample-y on free dim: Y profile per sample
                xsel = csb["u_t_full"][:, 0:IMG].rearrange(
                    "p (a c) -> p a c", c=16)[:, :, 7:9]
                for si in range(2):
                    b = b0 + si
                    at = ps_mm.tile([128, 2, 28], F32, tag="mm", name="at")
                    for h in range(2):
                        nc.tensor.matmul(
                            at[:, h, :],
                            tpair[:, si * IMG + h * H: si * IMG + h * H + 128],
                            xsel)
                    nc.vector.tensor_reduce(
                        Cxa[:, :, b], at[0:H, :, :],
                        axis=AX.X, op=OP.max)

        def tail_group(g):
            """Bounds, pads, boxes, interp coords for samples [g*GS, (g+1)*GS)."""
            sl = slice(g * GS, (g + 1) * GS)
            # NOTE: Ry (reduced over T'-free axis) is the X profile, Cx the Y
            # profile: the 14x14 grid reshape puts y on the j (row) index, which
            # lands on the partition axis of T', so the first amap matmul
            # upsamples x on partitions.
            Rt = st([GS, IMG], F32, f"Rt{g}")
            Ct = st([GS, IMG], F32, f"Ct{g}")
            for h in range(2):
                pt = ps_mm.tile([GS, H], F32, tag="mm", name="pt")
                nc.tensor.transpose(pt[:], Ry[h][:, sl], csb["identity112"][:])
                nc.scalar.copy(Rt[:, h * H:(h + 1) * H], pt[:])
                pt2 = ps_mm.tile([GS, H], F32, tag="mm", name="pt2")
                nc.tensor.transpose(pt2[:], Cxa[:, h, sl], csb["identity112"][:])
                nc.scalar.copy(Ct[:, h * H:(h + 1) * H], pt2[:])

            maxv = st([GS, 1], F32, f"maxv{g}")
            nc.vector.tensor_reduce(maxv[:], Rt[:], axis=AX.X, op=OP.max)
            tthr = st([GS, 1], F32, f"tthr{g}")
            nc.scalar.mul(tthr[:], maxv[:], 0.6)
            condf = st([GS, 1], F32, f"condf{g}")
            nc.vector.tensor_scalar(condf[:], maxv[:], 6e-6, None, op0=OP.is_lt)
            notc = st([GS, 1], F32, f"notc{g}")
            nc.vector.tensor_scalar(notc[:], condf[:], -1.0, 1.0,
                                    op0=OP.mult, op1=OP.add)

            # all bound arithmetic in f32 on integer values (exact below 2^21);
            # cast to int32 only at final box assembly.
            fbounds = {}
            box = st([GS, 4], I32, f"box{g}")
            for aname, R in (("x", Rt), ("y", Ct)):
                mf = st([GS, IMG], F32, f"mf{aname}{g}")
                nc.vector.tensor_scalar(mf[:], R[:], tthr[:], None, op0=OP.is_gt)
                cmin = st([GS, IMG], F32, f"cmin{aname}{g}")
                nc.vector.tensor_tensor(cmin[:], mf[:], csb["iota_mB"][0:GS, :],
                                        op=OP.mult)
                nc.vector.tensor_scalar(cmin[:], cmin[:], float(BIG), None, op0=OP.add)
                lo = st([GS, 1], F32, f"lo{aname}{g}")
                nc.vector.tensor_reduce(lo[:], cmin[:], axis=AX.X, op=OP.min)
                cmax = st([GS, IMG], F32, f"cmax{aname}{g}")
                nc.vector.tensor_tensor(cmax[:], mf[:], csb["iota_pB"][0:GS, :],
                                        op=OP.mult)
                nc.vector.tensor_scalar(cmax[:], cmax[:], -float(BIG), None, op0=OP.add)
                hi = st([GS, 1], F32, f"hi{aname}{g}")
                nc.vector.tensor_reduce(hi[:], cmax[:], axis=AX.X, op=OP.max)

                d = st([GS, 1], F32, f"d{aname}{g}")
                nc.vector.tensor_sub(d[:], hi[:], lo[:])
                prodf = st([GS, 1], F32, f"prodf{aname}{g}")
                nc.vector.tensor_scalar(prodf[:], d[:], 6554.0, None, op0=OP.mult)
                mle = st([GS, 22], F32, f"mle{aname}{g}")
                nc.vector.tensor_scalar(mle[:], csb["thr_pad"][0:GS, :], prodf[:],
                                        None, op0=OP.is_le)
                pad = st([GS, 1], F32, f"pad{aname}{g}")
                nc.vector.tensor_reduce(pad[:], mle[:], axis=AX.X, op=OP.add)
                lo1 = st([GS, 1], F32, f"lo1{aname}{g}")
                nc.vector.tensor_sub(lo1[:], lo[:], pad[:])
                nc.vector.tensor_scalar(lo1[:], lo1[:], 0.0, None, op0=OP.max)
                hi1 = st([GS, 1], F32, f"hi1{aname}{g}")
                nc.vector.tensor_add(hi1[:], hi[:], pad[:])
                nc.vector.tensor_scalar(hi1[:], hi1[:], float(IMG), None, op0=OP.min)
                # default box when maxv < 1e-6: lo=0, hi=IMG
                nc.vector.tensor_tensor(lo1[:], lo1[:], notc[:], op=OP.mult)
                nc.vector.tensor_tensor(hi1[:], hi1[:], notc[:], op=OP.mult)
                nc.vector.scalar_tensor_tensor(hi1[:], condf[:], float(IMG), hi1[:],
                                               op0=OP.mult, op1=OP.add)
                # degenerate guard: hi = max(hi, lo+1)
                lop = st([GS, 1], F32, f"lop{aname}{g}")
                nc.vector.tensor_scalar(lop[:], lo1[:], 1.0, None, op0=OP.add)
                nc.vector.tensor_tensor(hi1[:], hi1[:], lop[:], op=OP.max)
                ci = 0 if aname == "x" else 1
                nc.vector.tensor_copy(box[:, ci:ci + 1], lo1[:])
                nc.vector.tensor_copy(box[:, ci + 2:ci + 3], hi1[:])
                fbounds[aname] = (lo1, hi1)
            nc.sync.dma_start(out=bbox_o[sl, :], in_=box[:])

            # interp source coords g = lo + clip(s, 0, n-1), split into an
            # fp16-exact coarse part + small residual (keeps the PE-side
            # outer-difference fp16-accurate)
            for aname in ("y", "x"):
                lo_f, hi_f = fbounds[aname]
                n_f = st([GS, 1], F32, f"nf{aname}{g}")
                nc.vector.tensor_sub(n_f[:], hi_f[:], lo_f[:])
                scale = st([GS, 1], F32, f"scale{aname}{g}")
                nc.vector.tensor_scalar(scale[:], n_f[:], 1.0 / IMG, None, op0=OP.mult)
                upper = st([GS, 1], F32, f"upper{aname}{g}")
                nc.vector.tensor_scalar(upper[:], n_f[:], -1.0, None, op0=OP.add)
                sv = st([GS, IMG], F32, f"sv{aname}{g}")
                nc.scalar.activation(sv[:], csb["iota_half_f"][0:GS, :], ACTF.Relu,
                                     bias=biasm05[:], scale=scale[:])
                gv = st([GS, IMG], F32, f"gv{aname}{g}")
                nc.vector.tensor_scalar(gv[:], sv[:], upper[:], lo_f[:],
                                        op0=OP.min, op1=OP.add)
                ga = st([GS, IMG], F16, f"ga{aname}{g}")
                nc.vector.tensor_copy(ga[:], gv[:])          # coarse (fp16-rounded)
                ga32 = st([GS, IMG], F32, f"ga32{aname}{g}")
                nc.vector.tensor_copy(ga32[:], ga[:])
                gb = st([GS, IMG], F16, f"gb{aname}{g}")
                nc.vector.tensor_sub(gb[:], gv[:], ga32[:])  # residual, |.|<=0.03
                rv = r2[aname][:].rearrange("p (b o) -> p b o", o=NPAD)
                nc.sync.dma_start(out=rv[0:1, sl, 0:IMG], in_=ga[:])
                nc.sync.dma_start(out=rv[1:2, sl, 0:IMG], in_=gb[:])

        def interp_group(g):
            """W build + per-channel interp matmuls for samples [g*GS, (g+1)*GS)."""
            for b in range(g * GS, (g + 1) * GS):
                # W slabs: [ -Wy_t | -Wx_h ] as (112, 512) fp16, negated weights;
                # the sign cancels across the two interp matmul stages.
                Wh = []
                for h in range(2):
                    wp = ps_mm.tile([128, 2 * NPAD], F32, tag="mm", name="wp")
                    nc.tensor.matmul(
                        wp[:, 0:NPAD], csb[f"lhsT_wy{h}"][:],
                        r2["y"][:, b * NPAD:(b + 1) * NPAD])
                    nc.tensor.matmul(
                        wp[:, NPAD:2 * NPAD], csb[f"lhsT_wx{h}"][:],
                        r2["x"][:, b * NPAD:(b + 1) * NPAD])
                    wa = wabsp.tile([H, 2 * NPAD], F16, tag="wabs", name="wa")
                    nc.scalar.activation(wa[:], wp[0:H, :], ACTF.Abs)
                    wt = wpool.tile([H, 2 * NPAD], F16, tag="w", name="wt")
                    if b % 2 == 0:
                        nc.vector.tensor_scalar(wt[:], wa[:], 1.0, 1.0,
                                                op0=OP.min, op1=OP.subtract)
                    else:
                        nc.scalar.activation(wt[:], wa[:], ACTF.Relu,
                                             bias=1.0, scale=-1.0)
                    Wh.append(wt)

                for ch in range(3):
                    it = imgp.tile([H, 2 * NPAD], F16, tag="img", name="it")
                    base = (b * 3 + ch) * IMG * IMG
                    eng = nc.gpsimd if ch % 2 == 0 else nc.sync
                    eng.dma_start(
                        out=it[:, 0:464],
                        in_=bass.AP(tensor=xin[:].tensor, offset=base,
                                    ap=[[2 * IMG, H], [1, 464]]))
                    tp = ps_tmp.tile([128, 2 * NPAD], F32, tag="tmp", name="tp2")
                    for xh in range(2):
                        for t in range(2):
                            nc.tensor.matmul(
                                tp[:, xh * NPAD:(xh + 1) * NPAD],
                                it[:, t * IMG + xh * H: t * IMG + xh * H + 128],
                                Wh[t][:, 0:NPAD],
                                start=(t == 0), stop=(t == 1))
                    ts_ = tmpsb.tile([H, 2 * NPAD], F16, tag="tmpsb", name="ts_")
                    if ch % 2 == 0:
                        nc.scalar.copy(ts_[:], tp[0:H, :])
                    else:
                        nc.vector.tensor_copy(ts_[:], tp[0:H, :])
                    cp = ps_crop.tile([128, 2 * NPAD], F32, tag="crop", name="cp")
                    for t in range(2):
                        for xh in range(2):
                            nc.tensor.matmul(
                                cp[:, t * NPAD: t * NPAD + IMG],
                                ts_[:, xh * NPAD + t: xh * NPAD + t + 255: 2],
                                Wh[xh][:, NPAD: NPAD + IMG],
                                start=(xh == 0), stop=(xh == 1))
                    cs = cropsb.tile([H, 2, IMG], F32, tag="cropsb", name="cs")
                    cpv = cp[0:H, :].rearrange("p (t o) -> p t o", t=2)[:, :, 0:IMG]
                    if ch % 2 == 0:
                        nc.vector.tensor_copy(cs[:], cpv)
                    else:
                        nc.scalar.copy(cs[:], cpv)
                    eng2 = nc.sync if ch % 2 == 0 else nc.gpsimd
                    eng2.dma_start(
                        out=crops_o[b, ch].rearrange("(p two) x -> p two x", two=2),
                        in_=cs[:])

        bbox_group(0)
        tail_group(0)
        bbox_group(1)
        interp_group(0)
        tail_group(1)
        interp_group(1)

    if split_waits:
        _split_multi_waits(nc)
    nc.finalize()
    return nc, consts


_CACHE = {}


def _get_program(split_waits=True):
    key = ("prog", split_waits)
    if key not in _CACHE:
        _CACHE[key] = _build_program(split_waits)
    return _CACHE[key]


def _prearrange_attn(attn_slice):
    """(n, 6, 196) f32 -> (84, n, 14) fp16 in (head, j) x (sample, i) layout."""
    n = attn_slice.shape[0]
    a = attn_slice.reshape(n, HEADS, G, G).transpose(1, 2, 0, 3)
    return np.ascontiguousarray(a.reshape(HEADS * G, n, G).astype(np.float16))


def kernel(x, attn_weights):
    x = np.asarray(x)
    attn_weights = np.asarray(attn_weights)
    B = x.shape[0]
    per = B // NC_CORES
    assert per == PB, (B, PB)
    npatch = G * G
    x16 = np.ascontiguousarray(x, dtype=np.float16)
    attn_sl = attn_weights[:, :, 0, -npatch:]  # (B, 6, 196)

    nc, consts = _get_program()
    in_maps = []
    for i in range(NC_CORES):
        m = {k: v for k, v in consts.items()}
        xs = x16[i * per:(i + 1) * per].reshape(-1)
        m["xin"] = np.concatenate([xs, np.zeros(64, np.float16)])
        m["attn"] = _prearrange_attn(attn_sl[i * per:(i + 1) * per])
        in_maps.append(m)
    res = run_bass_kernel_spmd(nc, in_maps, list(range(NC_CORES))).results
    crops = np.concatenate([r["crops"] for r in res], axis=0)
    bboxes = np.concatenate([r["bboxes"] for r in res], axis=0).astype(np.int32)
    return crops, bboxes


# revision 48
# speedup vs baseline: 1.0149x; 1.0149x over previous
"""DinoSwav attention-crop kernel for Trainium2 (Bass/Tile), 8-core data parallel.

Per sample:
  1. CLS-attention map (mean over heads) -> 14x14, bilinear-upsampled to 224x224
  2. threshold mask (> 0.6*max), row/col any -> bbox with 10% margin
  3. crop + bilinear resize back to 224x224

Everything runs on device. The crop-resize is expressed as two fp16 matmuls per
channel with data-dependent interpolation matrices Wy/Wx built on-device from
the bbox via a rank-3 outer-difference matmul (int+frac coordinate split keeps
fp16 exact) + min(|d|,1)-1 (negated weights; the sign cancels across the two
matmul stages).
"""

import numpy as np
from contextlib import ExitStack

import concourse.bass as bass
import concourse.tile as tile
from concourse import mybir
from concourse.bass_utils import run_bass_kernel_spmd

F32 = mybir.dt.float32
F16 = mybir.dt.float16
I32 = mybir.dt.int32
AX = mybir.AxisListType
OP = mybir.AluOpType
ACTF = mybir.ActivationFunctionType

IMG = 224
G = 14          # patch grid
HEADS = 6
PB = 16         # samples per core
NC_CORES = 8
H = 112         # half of IMG (partition tile)
NPAD = 256      # padded matmul free dim; two blocks pack into one PSUM bank
BIG = 1 << 20   # sentinel for masked argmin/argmax; exact in f32


def _upsample_matrix():
    """U[x, j]: bilinear 14 -> 224 weights (half-pixel centers). Exact in fp16."""
    s = (np.arange(IMG, dtype=np.float64) + 0.5) * G / IMG - 0.5
    s = np.clip(s, 0.0, G - 1)
    U = np.maximum(0.0, 1.0 - np.abs(s[:, None] - np.arange(G)[None, :]))
    return U.astype(np.float32)  # (224, 14)


def _host_consts():
    U = _upsample_matrix()
    c = {}
    u_rep = np.zeros((HEADS * G, NPAD), np.float32)
    u_rep[:, :IMG] = np.tile(U.T, (HEADS, 1))
    c["u_rep"] = u_rep.astype(np.float16)
    for h in range(2):
        ut = np.zeros((G, 128), np.float32)
        ut[:, :H] = U[h * H:(h + 1) * H, :].T
        c[f"u_t_h{h}"] = ut.astype(np.float16)  # (14, 128): M=128 enables FWL
    u_t_full = np.zeros((G, NPAD), np.float32)
    u_t_full[:, :IMG] = U.T
    c["u_t_full"] = u_t_full.astype(np.float16)
    c["identity112"] = np.eye(H, dtype=np.float32)
    ar = np.arange(IMG, dtype=np.int64)
    c["iota_mB"] = np.tile((ar - BIG).astype(np.float32), (PB, 1))   # (16, 224)
    c["iota_pB"] = np.tile((ar + BIG).astype(np.float32), (PB, 1))
    c["iota_half_f"] = np.tile((ar + 0.5).astype(np.float32), (PB, 1))
    c["thr_pad"] = np.tile((np.arange(1, 23) * 65536.0).astype(np.float32), (PB, 1))
    for h in range(2):
        w = np.ones((3, 128), np.float32)    # rows: [gi, gf, -x], x = h*112+p
        w[2, :] = -1000.0
        w[2, :H] = -(h * H + np.arange(H, dtype=np.float32))
        c[f"lhsT_wx{h}"] = w.astype(np.float16)
    for t in range(2):
        w = np.ones((3, 128), np.float32)    # rows: [gi, gf, -y], y = 2p+t
        w[2, :] = -1000.0
        w[2, :H] = -(2.0 * np.arange(H, dtype=np.float32) + t)
        c[f"lhsT_wy{t}"] = w.astype(np.float16)
    r2i = np.zeros((3, PB * NPAD), np.float32)
    r2i[2, :] = 1.0
    c["r2_init"] = r2i.astype(np.float16)
    return c


def _split_multi_waits(nc, max_waits=1):
    """The walrus build in this environment accepts only one sync-wait per
    instruction; hoist extra waits onto same-engine NOPs placed just before."""
    ctr = 0
    for fn in nc.m.functions:
        for blk in fn.blocks:
            lst = blk.instructions
            out = []
            changed = False
            for ins in lst:
                si = ins.sync_info
                if si is not None and len(si.on_wait) > max_waits:
                    waits = list(si.on_wait)
                    hoist, keep = waits[:-max_waits], waits[-max_waits:]
                    for w in hoist:
                        ctr += 1
                        nop = mybir.InstNoOp(
                            name=f"waitsplit-{ctr}",
                            engine=ins.engine,
                            ins=[], outs=[],
                            sync_info=mybir.SyncInfo(on_wait=[w], on_update=[]),
                        )
                        out.append(nop)
                    si.on_wait = keep
                    changed = True
                out.append(ins)
            if changed:
                blk.instructions = out


def _build_program(split_waits=True):
    nc = bass.Bass()
    xin = nc.declare_dram_parameter("xin", [PB * 3 * IMG * IMG + 64], F16, isOutput=False)
    attn = nc.declare_dram_parameter("attn", [HEADS * G, PB, G], F16, isOutput=False)
    consts = _host_consts()
    cdecl = {}
    for name, arr in consts.items():
        dt = {np.dtype(np.float32): F32, np.dtype(np.float16): F16,
              np.dtype(np.int32): I32}[arr.dtype]
        cdecl[name] = nc.declare_dram_parameter(name, list(arr.shape), dt, isOutput=False)
    crops_o = nc.declare_dram_parameter("crops", [PB, 3, IMG, IMG], F16, isOutput=True)
    bbox_o = nc.declare_dram_parameter("bboxes", [PB, 4], I32, isOutput=True)
    warm_scr = nc.dram_tensor("warm_scr", [H, 2 * NPAD], F16)
    GS = 8  # bbox group size: lets interp(g) overlap bbox(g+1)

    with tile.TileContext(nc) as tc, ExitStack() as ctx:
        singles = ctx.enter_context(tc.tile_pool(name="singles", bufs=1))
        tpairp = ctx.enter_context(tc.tile_pool(name="tpair", bufs=3))
        wpool = ctx.enter_context(tc.tile_pool(name="wpool", bufs=6))
        wabsp = ctx.enter_context(tc.tile_pool(name="wabs", bufs=3))
        imgp = ctx.enter_context(tc.tile_pool(name="imgp", bufs=6))
        tmpsb = ctx.enter_context(tc.tile_pool(name="tmpsb", bufs=6))
        cropsb = ctx.enter_context(tc.tile_pool(name="cropsb", bufs=6))
        smallp = ctx.enter_context(tc.tile_pool(name="smallp", bufs=1))
        ps_mm = ctx.enter_context(tc.tile_pool(name="ps_mm", bufs=3, space="PSUM"))
        ps_tmp = ctx.enter_context(tc.tile_pool(name="ps_tmp", bufs=3, space="PSUM"))
        ps_crop = ctx.enter_context(tc.tile_pool(name="ps_crop", bufs=2, space="PSUM"))

        # ---- load constants into SBUF, round-robin across engine queues so
        #      the ~650ns-per-DMA issue cost is parallel, not serial ----
        csb = {}
        dma_engs = [nc.sync, nc.gpsimd]
        ei = 0
        for name, arr in consts.items():
            if name == "r2_init":
                continue
            t = singles.tile(list(arr.shape), cdecl[name].dtype, tag=name)
            dma_engs[ei % len(dma_engs)].dma_start(out=t[:], in_=cdecl[name][:])
            ei += 1
            csb[name] = t

        lhsT_a = singles.tile([HEADS * G, PB, G], F16, tag="lhsT_a")
        nc.sync.dma_start(out=lhsT_a[:], in_=attn[:])

        # ---- persistent small tensors ----
        Ry = [singles.tile([H, PB], F32, tag=f"Ry{h}", name=f"Ry{h}") for h in range(2)]
        Cxa = singles.tile([H, 2, PB], F32, tag="Cxa", name="Cxa")
        r2 = {a: singles.tile([3, PB * NPAD], F16, tag=f"r2{a}", name=f"r2{a}")
              for a in ("y", "x")}
        nc.sync.dma_start(out=r2["y"][:], in_=cdecl["r2_init"][:])
        nc.gpsimd.dma_start(out=r2["x"][:], in_=cdecl["r2_init"][:])

        # ---- PE warm-up: ~5us of dense matmuls releases the HAM clock gate ----
        wsrc = singles.tile([128, 2 * NPAD], F16, tag="wsrc")
        nc.vector.memset(wsrc[:], 1.0)
        wps = ps_mm.tile([128, 2 * NPAD], F32, tag="mm", name="wps")
        NWARM = 8
        for i in range(NWARM):
            nc.tensor.matmul(wps[:], wsrc[:, 0:128], wsrc[:],
                             start=(i == 0), stop=(i == NWARM - 1))
        wout = singles.tile([H, 2 * NPAD], F16, tag="wout")
        nc.scalar.copy(wout[:], wps[0:H, :])
        nc.sync.dma_start(out=warm_scr[:], in_=wout[:])

        def st(shape, dtype, tag):
            return smallp.tile(shape, dtype, tag=tag, name=tag)

        biasm05 = st([GS, 1], F32, "biasm05")
        nc.vector.memset(biasm05[:], -0.5)

        def bbox_group(g):
            """T' + amap/amapT matmuls + row/col maxes for samples [g*GS, (g+1)*GS)."""
            for b0 in range(g * GS, (g + 1) * GS, 2):
                tpair = tpairp.tile([G, 2 * NPAD], F16, tag="tpair", name="tpair")
                nc.vector.memset(tpair[:, 2 * IMG:2 * NPAD], 0.0)
                for si in range(2):
                    b = b0 + si
                    tp = ps_mm.tile([G, NPAD], F32, tag="mm", name="tp")
                    nc.tensor.matmul(tp[:, 0:IMG], lhsT_a[:, b, :],
                                     csb["u_rep"][:, 0:IMG])
                    nc.scalar.copy(tpair[:, si * IMG:(si + 1) * IMG], tp[:, 0:IMG])
                # upsample-x on partitions: X profile for both samples at once.
                # Along the reduced (y) axis the map is piecewise linear between
                # grid nodes at pixels 16i+7.5, so per-segment pixel maxima are
                # at columns {16i+7, 16i+8}: reducing over those 28 is exact.
                ysel = tpair[:, 0:2 * IMG].rearrange(
                    "p (s a c) -> p s a c", s=2, c=16)[:, :, :, 7:9]
                for h in range(2):
                    am = ps_mm.tile([128, 2 * 28], F32, tag="mm", name="am")
                    nc.tensor.matmul(am[:], csb[f"u_t_h{h}"][:], ysel)
                    nc.vector.tensor_reduce(
                        Ry[h][:, b0:b0 + 2],
                        am[0:H, :].rearrange("p (s c) -> p s c", s=2),
                        axis=AX.X, op=OP.max)
                # upsample-y on free dim: Y profile per sample
                xsel = csb["u_t_full"][:, 0:IMG].rearrange(
                    "p (a c) -> p a c", c=16)[:, :, 7:9]
                for si in range(2):
                    b = b0 + si
                    at = ps_mm.tile([128, 2, 28], F32, tag="mm", name="at")
                    for h in range(2):
                        nc.tensor.matmul(
                            at[:, h, :],
                            tpair[:, si * IMG + h * H: si * IMG + h * H + 128],
                            xsel)
                    nc.vector.tensor_reduce(
                        Cxa[:, :, b], at[0:H, :, :],
                        axis=AX.X, op=OP.max)

        def tail_group(g):
            """Bounds, pads, boxes, interp coords for samples [g*GS, (g+1)*GS)."""
            sl = slice(g * GS, (g + 1) * GS)
            # NOTE: Ry (reduced over T'-free axis) is the X profile, Cx the Y
            # profile: the 14x14 grid reshape puts y on the j (row) index, which
            # lands on the partition axis of T', so the first amap matmul
            # upsamples x on partitions.
            Rt = st([GS, IMG], F32, f"Rt{g}")
            Ct = st([GS, IMG], F32, f"Ct{g}")
            for h in range(2):
                pt = ps_mm.tile([GS, H], F32, tag="mm", name="pt")
                nc.tensor.transpose(pt[:], Ry[h][:, sl], csb["identity112"][:])
                nc.scalar.copy(Rt[:, h * H:(h + 1) * H], pt[:])
                pt2 = ps_mm.tile([GS, H], F32, tag="mm", name="pt2")
                nc.tensor.transpose(pt2[:], Cxa[:, h, sl], csb["identity112"][:])
                nc.scalar.copy(Ct[:, h * H:(h + 1) * H], pt2[:])

            maxv = st([GS, 1], F32, f"maxv{g}")
            nc.vector.tensor_reduce(maxv[:], Rt[:], axis=AX.X, op=OP.max)
            tthr = st([GS, 1], F32, f"tthr{g}")
            nc.scalar.mul(tthr[:], maxv[:], 0.6)
            condf = st([GS, 1], F32, f"condf{g}")
            nc.vector.tensor_scalar(condf[:], maxv[:], 6e-6, None, op0=OP.is_lt)
            notc = st([GS, 1], F32, f"notc{g}")
            nc.vector.tensor_scalar(notc[:], condf[:], -1.0, 1.0,
                                    op0=OP.mult, op1=OP.add)

            # all bound arithmetic in f32 on integer values (exact below 2^21);
            # cast to int32 only at final box assembly.
            fbounds = {}
            box = st([GS, 4], I32, f"box{g}")
            for aname, R in (("x", Rt), ("y", Ct)):
                mf = st([GS, IMG], F32, f"mf{aname}{g}")
                nc.vector.tensor_scalar(mf[:], R[:], tthr[:], None, op0=OP.is_gt)
                cmin = st([GS, IMG], F32, f"cmin{aname}{g}")
                nc.vector.tensor_tensor(cmin[:], mf[:], csb["iota_mB"][0:GS, :],
                                        op=OP.mult)
                nc.vector.tensor_scalar(cmin[:], cmin[:], float(BIG), None, op0=OP.add)
                lo = st([GS, 1], F32, f"lo{aname}{g}")
                nc.vector.tensor_reduce(lo[:], cmin[:], axis=AX.X, op=OP.min)
                cmax = st([GS, IMG], F32, f"cmax{aname}{g}")
                nc.vector.tensor_tensor(cmax[:], mf[:], csb["iota_pB"][0:GS, :],
                                        op=OP.mult)
                nc.vector.tensor_scalar(cmax[:], cmax[:], -float(BIG), None, op0=OP.add)
                hi = st([GS, 1], F32, f"hi{aname}{g}")
                nc.vector.tensor_reduce(hi[:], cmax[:], axis=AX.X, op=OP.max)

                d = st([GS, 1], F32, f"d{aname}{g}")
                nc.vector.tensor_sub(d[:], hi[:], lo[:])
                prodf = st([GS, 1], F32, f"prodf{aname}{g}")
                nc.vector.tensor_scalar(prodf[:], d[:], 6554.0, None, op0=OP.mult)
                mle = st([GS, 22], F32, f"mle{aname}{g}")
                nc.vector.tensor_scalar(mle[:], csb["thr_pad"][0:GS, :], prodf[:],
                                        None, op0=OP.is_le)
                pad = st([GS, 1], F32, f"pad{aname}{g}")
                nc.vector.tensor_reduce(pad[:], mle[:], axis=AX.X, op=OP.add)
                lo1 = st([GS, 1], F32, f"lo1{aname}{g}")
                nc.vector.tensor_sub(lo1[:], lo[:], pad[:])
                nc.vector.tensor_scalar(lo1[:], lo1[:], 0.0, None, op0=OP.max)
                hi1 = st([GS, 1], F32, f"hi1{aname}{g}")
                nc.vector.tensor_add(hi1[:], hi[:], pad[:])
                nc.vector.tensor_scalar(hi1[:], hi1[:], float(IMG), None, op0=OP.min)
                # default box when maxv < 1e-6: lo=0, hi=IMG
                nc.vector.tensor_tensor(lo1[:], lo1[:], notc[:], op=OP.mult)
                nc.vector.tensor_tensor(hi1[:], hi1[:], notc[:], op=OP.mult)
                nc.vector.scalar_tensor_tensor(hi1[:], condf[:], float(IMG), hi1[:],
                                               op0=OP.mult, op1=OP.add)
                # degenerate guard: hi = max(hi, lo+1)
                lop = st([GS, 1], F32, f"lop{aname}{g}")
                nc.vector.tensor_scalar(lop[:], lo1[:], 1.0, None, op0=OP.add)
                nc.vector.tensor_tensor(hi1[:], hi1[:], lop[:], op=OP.max)
                ci = 0 if aname == "x" else 1
                nc.vector.tensor_copy(box[:, ci:ci + 1], lo1[:])
                nc.vector.tensor_copy(box[:, ci + 2:ci + 3], hi1[:])
                fbounds[aname] = (lo1, hi1)
            nc.sync.dma_start(out=bbox_o[sl, :], in_=box[:])

            # interp source coords g = lo + clip(s, 0, n-1), split into an
            # fp16-exact coarse part + small residual (keeps the PE-side
            # outer-difference fp16-accurate)
            for aname in ("y", "x"):
                lo_f, hi_f = fbounds[aname]
                n_f = st([GS, 1], F32, f"nf{aname}{g}")
                nc.vector.tensor_sub(n_f[:], hi_f[:], lo_f[:])
                scale = st([GS, 1], F32, f"scale{aname}{g}")
                nc.vector.tensor_scalar(scale[:], n_f[:], 1.0 / IMG, None, op0=OP.mult)
                upper = st([GS, 1], F32, f"upper{aname}{g}")
                nc.vector.tensor_scalar(upper[:], n_f[:], -1.0, None, op0=OP.add)
                sv = st([GS, IMG], F32, f"sv{aname}{g}")
                nc.scalar.activation(sv[:], csb["iota_half_f"][0:GS, :], ACTF.Relu,
                                     bias=biasm05[:], scale=scale[:])
                gv = st([GS, IMG], F32, f"gv{aname}{g}")
                nc.vector.tensor_scalar(gv[:], sv[:], upper[:], lo_f[:],
                                        op0=OP.min, op1=OP.add)
                ga = st([GS, IMG], F16, f"ga{aname}{g}")
                nc.vector.tensor_copy(ga[:], gv[:])          # coarse (fp16-rounded)
                ga32 = st([GS, IMG], F32, f"ga32{aname}{g}")
                nc.vector.tensor_copy(ga32[:], ga[:])
                gb = st([GS, IMG], F16, f"gb{aname}{g}")
                nc.vector.tensor_sub(gb[:], gv[:], ga32[:])  # residual, |.|<=0.03
                rv = r2[aname][:].rearrange("p (b o) -> p b o", o=NPAD)
                nc.sync.dma_start(out=rv[0:1, sl, 0:IMG], in_=ga[:])
                nc.sync.dma_start(out=rv[1:2, sl, 0:IMG], in_=gb[:])

        def interp_group(g):
            """W build + per-channel interp matmuls for samples [g*GS, (g+1)*GS)."""
            for b in range(g * GS, (g + 1) * GS):
                # W slabs: [ -Wy_t | -Wx_h ] as (112, 512) fp16, negated weights;
                # the sign cancels across the two interp matmul stages.
                Wh = []
                for h in range(2):
                    wp = ps_mm.tile([128, 2 * NPAD], F32, tag="mm", name="wp")
                    nc.tensor.matmul(
                        wp[:, 0:NPAD], csb[f"lhsT_wy{h}"][:],
                        r2["y"][:, b * NPAD:(b + 1) * NPAD])
                    nc.tensor.matmul(
                        wp[:, NPAD:2 * NPAD], csb[f"lhsT_wx{h}"][:],
                        r2["x"][:, b * NPAD:(b + 1) * NPAD])
                    wa = wabsp.tile([H, 2 * NPAD], F16, tag="wabs", name="wa")
                    nc.scalar.activation(wa[:], wp[0:H, :], ACTF.Abs)
                    wt = wpool.tile([H, 2 * NPAD], F16, tag="w", name="wt")
                    if b % 2 == 0:
                        nc.vector.tensor_scalar(wt[:], wa[:], 1.0, 1.0,
                                                op0=OP.min, op1=OP.subtract)
                    else:
                        nc.scalar.activation(wt[:], wa[:], ACTF.Relu,
                                             bias=1.0, scale=-1.0)
                    Wh.append(wt)

                for ch in range(3):
                    it = imgp.tile([H, 2 * NPAD], F16, tag="img", name="it")
                    base = (b * 3 + ch) * IMG * IMG
                    eng = nc.gpsimd if ch % 2 == 0 else nc.sync
                    eng.dma_start(
                        out=it[:, 0:464],
                        in_=bass.AP(tensor=xin[:].tensor, offset=base,
                                    ap=[[2 * IMG, H], [1, 464]]))
                    tp = ps_tmp.tile([128, 2 * NPAD], F32, tag="tmp", name="tp2")
                    for xh in range(2):
                        for t in range(2):
                            nc.tensor.matmul(
                                tp[:, xh * NPAD:(xh + 1) * NPAD],
                                it[:, t * IMG + xh * H: t * IMG + xh * H + 128],
                                Wh[t][:, 0:NPAD],
                                start=(t == 0), stop=(t == 1))
                    ts_ = tmpsb.tile([H, 2 * NPAD], F16, tag="tmpsb", name="ts_")
                    if ch % 2 == 0:
                        nc.scalar.copy(ts_[:], tp[0:H, :])
                    else:
                        nc.vector.tensor_copy(ts_[:], tp[0:H, :])
                    cp = ps_crop.tile([128, 2 * NPAD], F32, tag="crop", name="cp")
                    for t in range(2):
                        for xh in range(2):
                            nc.tensor.matmul(
                                cp[:, t * NPAD: t * NPAD + IMG],
                                ts_[:, xh * NPAD + t: xh * NPAD + t + 255: 2],
                                Wh[xh][:, NPAD: NPAD + IMG],
                                start=(xh == 0), stop=(xh == 1))
                    cs = cropsb.tile([H, 2, IMG], F16, tag="cropsb", name="cs")
                    cpv = cp[0:H, :].rearrange("p (t o) -> p t o", t=2)[:, :, 0:IMG]
                    if ch % 2 == 0:
                        nc.vector.tensor_copy(cs[:], cpv)
                    else:
                        nc.scalar.copy(cs[:], cpv)
                    eng2 = nc.sync if ch % 2 == 0 else nc.gpsimd
                    eng2.dma_start(
                        out=crops_o[b, ch].rearrange("(p two) x -> p two x", two=2),
                        in_=cs[:])

        bbox_group(0)
        tail_group(0)
        bbox_group(1)
        tail_group(1)
        interp_group(0)
        interp_group(1)

    if split_waits:
        _split_multi_waits(nc)
    nc.finalize()
    return nc, consts


_CACHE = {}


def _get_program(split_waits=True):
    key = ("prog", split_waits)
    if key not in _CACHE:
        _CACHE[key] = _build_program(split_waits)
    return _CACHE[key]


def _prearrange_attn(attn_slice):
    """(n, 6, 196) f32 -> (84, n, 14) fp16 in (head, j) x (sample, i) layout."""
    n = attn_slice.shape[0]
    a = attn_slice.reshape(n, HEADS, G, G).transpose(1, 2, 0, 3)
    return np.ascontiguousarray(a.reshape(HEADS * G, n, G).astype(np.float16))


def kernel(x, attn_weights):
    x = np.asarray(x)
    attn_weights = np.asarray(attn_weights)
    B = x.shape[0]
    per = B // NC_CORES
    assert per == PB, (B, PB)
    npatch = G * G
    x16 = np.ascontiguousarray(x, dtype=np.float16)
    attn_sl = attn_weights[:, :, 0, -npatch:]  # (B, 6, 196)

    nc, consts = _get_program()
    in_maps = []
    for i in range(NC_CORES):
        m = {k: v for k, v in consts.items()}
        xs = x16[i * per:(i + 1) * per].reshape(-1)
        m["xin"] = np.concatenate([xs, np.zeros(64, np.float16)])
        m["attn"] = _prearrange_attn(attn_sl[i * per:(i + 1) * per])
        in_maps.append(m)
    res = run_bass_kernel_spmd(nc, in_maps, list(range(NC_CORES))).results
    crops = np.concatenate([r["crops"] for r in res], axis=0).astype(np.float32)
    bboxes = np.concatenate([r["bboxes"] for r in res], axis=0).astype(np.int32)
    return crops, bboxes


# revision 49
# speedup vs baseline: 1.0288x; 1.0137x over previous
"""DinoSwav attention-crop kernel for Trainium2 (Bass/Tile), 8-core data parallel.

Per sample:
  1. CLS-attention map (mean over heads) -> 14x14, bilinear-upsampled to 224x224
  2. threshold mask (> 0.6*max), row/col any -> bbox with 10% margin
  3. crop + bilinear resize back to 224x224

Everything runs on device. The crop-resize is expressed as two fp16 matmuls per
channel with data-dependent interpolation matrices Wy/Wx built on-device from
the bbox via a rank-3 outer-difference matmul (int+frac coordinate split keeps
fp16 exact) + min(|d|,1)-1 (negated weights; the sign cancels across the two
matmul stages).
"""

import numpy as np
from contextlib import ExitStack

import concourse.bass as bass
import concourse.tile as tile
from concourse import mybir
from concourse.bass_utils import run_bass_kernel_spmd

F32 = mybir.dt.float32
F16 = mybir.dt.float16
I32 = mybir.dt.int32
AX = mybir.AxisListType
OP = mybir.AluOpType
ACTF = mybir.ActivationFunctionType

IMG = 224
G = 14          # patch grid
HEADS = 6
PB = 16         # samples per core
NC_CORES = 8
H = 112         # half of IMG (partition tile)
NPAD = 256      # padded matmul free dim; two blocks pack into one PSUM bank
BIG = 1 << 20   # sentinel for masked argmin/argmax; exact in f32


def _upsample_matrix():
    """U[x, j]: bilinear 14 -> 224 weights (half-pixel centers). Exact in fp16."""
    s = (np.arange(IMG, dtype=np.float64) + 0.5) * G / IMG - 0.5
    s = np.clip(s, 0.0, G - 1)
    U = np.maximum(0.0, 1.0 - np.abs(s[:, None] - np.arange(G)[None, :]))
    return U.astype(np.float32)  # (224, 14)


def _host_consts():
    U = _upsample_matrix()
    c = {}
    u_rep = np.zeros((HEADS * G, NPAD), np.float32)
    u_rep[:, :IMG] = np.tile(U.T, (HEADS, 1))
    c["u_rep"] = u_rep.astype(np.float16)
    for h in range(2):
        ut = np.zeros((G, 128), np.float32)
        ut[:, :H] = U[h * H:(h + 1) * H, :].T
        c[f"u_t_h{h}"] = ut.astype(np.float16)  # (14, 128): M=128 enables FWL
    u_t_full = np.zeros((G, NPAD), np.float32)
    u_t_full[:, :IMG] = U.T
    c["u_t_full"] = u_t_full.astype(np.float16)
    c["identity112"] = np.eye(H, dtype=np.float32)
    ar = np.arange(IMG, dtype=np.int64)
    blob16 = np.concatenate([
        np.tile((ar - BIG).astype(np.float32), (PB, 1)),
        np.tile((ar + BIG).astype(np.float32), (PB, 1)),
        np.tile((ar + 0.5).astype(np.float32), (PB, 1)),
        np.tile((np.arange(1, 23) * 65536.0).astype(np.float32), (PB, 1)),
    ], axis=1)
    c["blob16"] = blob16  # iota_mB | iota_pB | iota_half_f | thr_pad
    blob3 = []
    for h in range(2):
        w = np.ones((3, 128), np.float32)    # rows: [gi, gf, -x], x = h*112+p
        w[2, :] = -1000.0
        w[2, :H] = -(h * H + np.arange(H, dtype=np.float32))
        blob3.append(w)
    for t in range(2):
        w = np.ones((3, 128), np.float32)    # rows: [gi, gf, -y], y = 2p+t
        w[2, :] = -1000.0
        w[2, :H] = -(2.0 * np.arange(H, dtype=np.float32) + t)
        blob3.append(w)
    c["blob3"] = np.concatenate(blob3, axis=1).astype(np.float16)
    r2i = np.zeros((3, PB * NPAD), np.float32)
    r2i[2, :] = 1.0
    c["r2_init"] = r2i.astype(np.float16)
    return c


def _split_multi_waits(nc, max_waits=1):
    """The walrus build in this environment accepts only one sync-wait per
    instruction; hoist extra waits onto same-engine NOPs placed just before."""
    ctr = 0
    for fn in nc.m.functions:
        for blk in fn.blocks:
            lst = blk.instructions
            out = []
            changed = False
            for ins in lst:
                si = ins.sync_info
                if si is not None and len(si.on_wait) > max_waits:
                    waits = list(si.on_wait)
                    hoist, keep = waits[:-max_waits], waits[-max_waits:]
                    for w in hoist:
                        ctr += 1
                        nop = mybir.InstNoOp(
                            name=f"waitsplit-{ctr}",
                            engine=ins.engine,
                            ins=[], outs=[],
                            sync_info=mybir.SyncInfo(on_wait=[w], on_update=[]),
                        )
                        out.append(nop)
                    si.on_wait = keep
                    changed = True
                out.append(ins)
            if changed:
                blk.instructions = out


def _build_program(split_waits=True):
    nc = bass.Bass()
    xin = nc.declare_dram_parameter("xin", [PB * 3 * IMG * IMG + 64], F16, isOutput=False)
    attn = nc.declare_dram_parameter("attn", [HEADS * G, PB, G], F16, isOutput=False)
    consts = _host_consts()
    cdecl = {}
    for name, arr in consts.items():
        dt = {np.dtype(np.float32): F32, np.dtype(np.float16): F16,
              np.dtype(np.int32): I32}[arr.dtype]
        cdecl[name] = nc.declare_dram_parameter(name, list(arr.shape), dt, isOutput=False)
    crops_o = nc.declare_dram_parameter("crops", [PB, 3, IMG, IMG], F16, isOutput=True)
    bbox_o = nc.declare_dram_parameter("bboxes", [PB, 4], I32, isOutput=True)
    warm_scr = nc.dram_tensor("warm_scr", [H, 2 * NPAD], F16)
    GS = 8  # bbox group size: lets interp(g) overlap bbox(g+1)

    with tile.TileContext(nc) as tc, ExitStack() as ctx:
        singles = ctx.enter_context(tc.tile_pool(name="singles", bufs=1))
        tpairp = ctx.enter_context(tc.tile_pool(name="tpair", bufs=3))
        wpool = ctx.enter_context(tc.tile_pool(name="wpool", bufs=6))
        wabsp = ctx.enter_context(tc.tile_pool(name="wabs", bufs=3))
        imgp = ctx.enter_context(tc.tile_pool(name="imgp", bufs=6))
        tmpsb = ctx.enter_context(tc.tile_pool(name="tmpsb", bufs=6))
        cropsb = ctx.enter_context(tc.tile_pool(name="cropsb", bufs=6))
        smallp = ctx.enter_context(tc.tile_pool(name="smallp", bufs=1))
        ps_mm = ctx.enter_context(tc.tile_pool(name="ps_mm", bufs=3, space="PSUM"))
        ps_tmp = ctx.enter_context(tc.tile_pool(name="ps_tmp", bufs=3, space="PSUM"))
        ps_crop = ctx.enter_context(tc.tile_pool(name="ps_crop", bufs=2, space="PSUM"))

        # ---- load constants into SBUF, round-robin across engine queues so
        #      the ~650ns-per-DMA issue cost is parallel, not serial ----
        csb = {}
        dma_engs = [nc.sync, nc.gpsimd]
        ei = 0
        for name, arr in consts.items():
            if name == "r2_init":
                continue
            t = singles.tile(list(arr.shape), cdecl[name].dtype, tag=name)
            dma_engs[ei % len(dma_engs)].dma_start(out=t[:], in_=cdecl[name][:])
            ei += 1
            csb[name] = t
        for i, nm in enumerate(("iota_mB", "iota_pB", "iota_half_f")):
            csb[nm] = csb["blob16"][:, i * IMG:(i + 1) * IMG]
        csb["thr_pad"] = csb["blob16"][:, 3 * IMG:3 * IMG + 22]
        csb["lhsT_wx0"] = csb["blob3"][:, 0:128]
        csb["lhsT_wx1"] = csb["blob3"][:, 128:256]
        csb["lhsT_wy0"] = csb["blob3"][:, 256:384]
        csb["lhsT_wy1"] = csb["blob3"][:, 384:512]

        lhsT_a = singles.tile([HEADS * G, PB, G], F16, tag="lhsT_a")
        nc.sync.dma_start(out=lhsT_a[:], in_=attn[:])

        # ---- persistent small tensors ----
        Ry = [singles.tile([H, PB], F32, tag=f"Ry{h}", name=f"Ry{h}") for h in range(2)]
        Cxa = singles.tile([H, 2, PB], F32, tag="Cxa", name="Cxa")
        r2 = {a: singles.tile([3, PB * NPAD], F16, tag=f"r2{a}", name=f"r2{a}")
              for a in ("y", "x")}
        nc.sync.dma_start(out=r2["y"][:], in_=cdecl["r2_init"][:])
        nc.gpsimd.dma_start(out=r2["x"][:], in_=cdecl["r2_init"][:])

        # ---- PE warm-up: ~5us of dense matmuls releases the HAM clock gate ----
        wsrc = singles.tile([128, 2 * NPAD], F16, tag="wsrc")
        nc.vector.memset(wsrc[:], 1.0)
        wps = ps_mm.tile([128, 2 * NPAD], F32, tag="mm", name="wps")
        NWARM = 8
        for i in range(NWARM):
            nc.tensor.matmul(wps[:], wsrc[:, 0:128], wsrc[:],
                             start=(i == 0), stop=(i == NWARM - 1))
        wout = singles.tile([H, 2 * NPAD], F16, tag="wout")
        nc.scalar.copy(wout[:], wps[0:H, :])
        nc.sync.dma_start(out=warm_scr[:], in_=wout[:])

        def st(shape, dtype, tag):
            return smallp.tile(shape, dtype, tag=tag, name=tag)

        biasm05 = st([GS, 1], F32, "biasm05")
        nc.vector.memset(biasm05[:], -0.5)

        def bbox_group(g):
            """T' + amap/amapT matmuls + row/col maxes for samples [g*GS, (g+1)*GS)."""
            for b0 in range(g * GS, (g + 1) * GS, 2):
                tpair = tpairp.tile([G, 2 * NPAD], F16, tag="tpair", name="tpair")
                nc.vector.memset(tpair[:, 2 * IMG:2 * NPAD], 0.0)
                for si in range(2):
                    b = b0 + si
                    tp = ps_mm.tile([G, NPAD], F32, tag="mm", name="tp")
                    nc.tensor.matmul(tp[:, 0:IMG], lhsT_a[:, b, :],
                                     csb["u_rep"][:, 0:IMG])
                    nc.scalar.copy(tpair[:, si * IMG:(si + 1) * IMG], tp[:, 0:IMG])
                # upsample-x on partitions: X profile for both samples at once.
                # Along the reduced (y) axis the map is piecewise linear between
                # grid nodes at pixels 16i+7.5, so per-segment pixel maxima are
                # at columns {16i+7, 16i+8}: reducing over those 28 is exact.
                ysel = tpair[:, 0:2 * IMG].rearrange(
                    "p (s a c) -> p s a c", s=2, c=16)[:, :, :, 7:9]
                for h in range(2):
                    am = ps_mm.tile([128, 2 * 28], F32, tag="mm", name="am")
                    nc.tensor.matmul(am[:], csb[f"u_t_h{h}"][:], ysel)
                    nc.vector.tensor_reduce(
                        Ry[h][:, b0:b0 + 2],
                        am[0:H, :].rearrange("p (s c) -> p s c", s=2),
                        axis=AX.X, op=OP.max)
                # upsample-y on free dim: Y profile per sample
                xsel = csb["u_t_full"][:, 0:IMG].rearrange(
                    "p (a c) -> p a c", c=16)[:, :, 7:9]
                for si in range(2):
                    b = b0 + si
                    at = ps_mm.tile([128, 2, 28], F32, tag="mm", name="at")
                    for h in range(2):
                        nc.tensor.matmul(
                            at[:, h, :],
                            tpair[:, si * IMG + h * H: si * IMG + h * H + 128],
                            xsel)
                    nc.vector.tensor_reduce(
                        Cxa[:, :, b], at[0:H, :, :],
                        axis=AX.X, op=OP.max)

        def tail_group(g):
            """Bounds, pads, boxes, interp coords for samples [g*GS, (g+1)*GS)."""
            sl = slice(g * GS, (g + 1) * GS)
            # NOTE: Ry (reduced over T'-free axis) is the X profile, Cx the Y
            # profile: the 14x14 grid reshape puts y on the j (row) index, which
            # lands on the partition axis of T', so the first amap matmul
            # upsamples x on partitions.
            Rt = st([GS, IMG], F32, f"Rt{g}")
            Ct = st([GS, IMG], F32, f"Ct{g}")
            for h in range(2):
                pt = ps_mm.tile([GS, H], F32, tag="mm", name="pt")
                nc.tensor.transpose(pt[:], Ry[h][:, sl], csb["identity112"][:])
                nc.scalar.copy(Rt[:, h * H:(h + 1) * H], pt[:])
                pt2 = ps_mm.tile([GS, H], F32, tag="mm", name="pt2")
                nc.tensor.transpose(pt2[:], Cxa[:, h, sl], csb["identity112"][:])
                nc.scalar.copy(Ct[:, h * H:(h + 1) * H], pt2[:])

            maxv = st([GS, 1], F32, f"maxv{g}")
            nc.vector.tensor_reduce(maxv[:], Rt[:], axis=AX.X, op=OP.max)
            tthr = st([GS, 1], F32, f"tthr{g}")
            nc.scalar.mul(tthr[:], maxv[:], 0.6)
            condf = st([GS, 1], F32, f"condf{g}")
            nc.vector.tensor_scalar(condf[:], maxv[:], 6e-6, None, op0=OP.is_lt)
            notc = st([GS, 1], F32, f"notc{g}")
            nc.vector.tensor_scalar(notc[:], condf[:], -1.0, 1.0,
                                    op0=OP.mult, op1=OP.add)

            # all bound arithmetic in f32 on integer values (exact below 2^21);
            # cast to int32 only at final box assembly.
            fbounds = {}
            box = st([GS, 4], I32, f"box{g}")
            for aname, R in (("x", Rt), ("y", Ct)):
                mf = st([GS, IMG], F32, f"mf{aname}{g}")
                nc.vector.tensor_scalar(mf[:], R[:], tthr[:], None, op0=OP.is_gt)
                cmin = st([GS, IMG], F32, f"cmin{aname}{g}")
                nc.vector.tensor_tensor(cmin[:], mf[:], csb["iota_mB"][0:GS, :],
                                        op=OP.mult)
                nc.vector.tensor_scalar(cmin[:], cmin[:], float(BIG), None, op0=OP.add)
                lo = st([GS, 1], F32, f"lo{aname}{g}")
                nc.vector.tensor_reduce(lo[:], cmin[:], axis=AX.X, op=OP.min)
                cmax = st([GS, IMG], F32, f"cmax{aname}{g}")
                nc.vector.tensor_tensor(cmax[:], mf[:], csb["iota_pB"][0:GS, :],
                                        op=OP.mult)
                nc.vector.tensor_scalar(cmax[:], cmax[:], -float(BIG), None, op0=OP.add)
                hi = st([GS, 1], F32, f"hi{aname}{g}")
                nc.vector.tensor_reduce(hi[:], cmax[:], axis=AX.X, op=OP.max)

                d = st([GS, 1], F32, f"d{aname}{g}")
                nc.vector.tensor_sub(d[:], hi[:], lo[:])
                prodf = st([GS, 1], F32, f"prodf{aname}{g}")
                nc.vector.tensor_scalar(prodf[:], d[:], 6554.0, None, op0=OP.mult)
                mle = st([GS, 22], F32, f"mle{aname}{g}")
                nc.vector.tensor_scalar(mle[:], csb["thr_pad"][0:GS, :], prodf[:],
                                        None, op0=OP.is_le)
                pad = st([GS, 1], F32, f"pad{aname}{g}")
                nc.vector.tensor_reduce(pad[:], mle[:], axis=AX.X, op=OP.add)
                lo1 = st([GS, 1], F32, f"lo1{aname}{g}")
                nc.vector.tensor_sub(lo1[:], lo[:], pad[:])
                nc.vector.tensor_scalar(lo1[:], lo1[:], 0.0, None, op0=OP.max)
                hi1 = st([GS, 1], F32, f"hi1{aname}{g}")
                nc.vector.tensor_add(hi1[:], hi[:], pad[:])
                nc.vector.tensor_scalar(hi1[:], hi1[:], float(IMG), None, op0=OP.min)
                # default box when maxv < 1e-6: lo=0, hi=IMG
                nc.vector.tensor_tensor(lo1[:], lo1[:], notc[:], op=OP.mult)
                nc.vector.tensor_tensor(hi1[:], hi1[:], notc[:], op=OP.mult)
                nc.vector.scalar_tensor_tensor(hi1[:], condf[:], float(IMG), hi1[:],
                                               op0=OP.mult, op1=OP.add)
                # degenerate guard: hi = max(hi, lo+1)
                lop = st([GS, 1], F32, f"lop{aname}{g}")
                nc.vector.tensor_scalar(lop[:], lo1[:], 1.0, None, op0=OP.add)
                nc.vector.tensor_tensor(hi1[:], hi1[:], lop[:], op=OP.max)
                ci = 0 if aname == "x" else 1
                nc.vector.tensor_copy(box[:, ci:ci + 1], lo1[:])
                nc.vector.tensor_copy(box[:, ci + 2:ci + 3], hi1[:])
                fbounds[aname] = (lo1, hi1)
            nc.sync.dma_start(out=bbox_o[sl, :], in_=box[:])

            # interp source coords g = lo + clip(s, 0, n-1), split into an
            # fp16-exact coarse part + small residual (keeps the PE-side
            # outer-difference fp16-accurate)
            for aname in ("y", "x"):
                lo_f, hi_f = fbounds[aname]
                n_f = st([GS, 1], F32, f"nf{aname}{g}")
                nc.vector.tensor_sub(n_f[:], hi_f[:], lo_f[:])
                scale = st([GS, 1], F32, f"scale{aname}{g}")
                nc.vector.tensor_scalar(scale[:], n_f[:], 1.0 / IMG, None, op0=OP.mult)
                upper = st([GS, 1], F32, f"upper{aname}{g}")
                nc.vector.tensor_scalar(upper[:], n_f[:], -1.0, None, op0=OP.add)
                sv = st([GS, IMG], F32, f"sv{aname}{g}")
                nc.scalar.activation(sv[:], csb["iota_half_f"][0:GS, :], ACTF.Relu,
                                     bias=biasm05[:], scale=scale[:])
                gv = st([GS, IMG], F32, f"gv{aname}{g}")
                nc.vector.tensor_scalar(gv[:], sv[:], upper[:], lo_f[:],
                                        op0=OP.min, op1=OP.add)
                ga = st([GS, IMG], F16, f"ga{aname}{g}")
                nc.vector.tensor_copy(ga[:], gv[:])          # coarse (fp16-rounded)
                ga32 = st([GS, IMG], F32, f"ga32{aname}{g}")
                nc.vector.tensor_copy(ga32[:], ga[:])
                gb = st([GS, IMG], F16, f"gb{aname}{g}")
                nc.vector.tensor_sub(gb[:], gv[:], ga32[:])  # residual, |.|<=0.03
                rv = r2[aname][:].rearrange("p (b o) -> p b o", o=NPAD)
                nc.sync.dma_start(out=rv[0:1, sl, 0:IMG], in_=ga[:])
                nc.sync.dma_start(out=rv[1:2, sl, 0:IMG], in_=gb[:])

        def interp_group(g):
            """W build + per-channel interp matmuls for samples [g*GS, (g+1)*GS)."""
            for b in range(g * GS, (g + 1) * GS):
                # W slabs: [ -Wy_t | -Wx_h ] as (112, 512) fp16, negated weights;
                # the sign cancels across the two interp matmul stages.
                Wh = []
                for h in range(2):
                    wp = ps_mm.tile([128, 2 * NPAD], F32, tag="mm", name="wp")
                    nc.tensor.matmul(
                        wp[:, 0:NPAD], csb[f"lhsT_wy{h}"][:],
                        r2["y"][:, b * NPAD:(b + 1) * NPAD])
                    nc.tensor.matmul(
                        wp[:, NPAD:2 * NPAD], csb[f"lhsT_wx{h}"][:],
                        r2["x"][:, b * NPAD:(b + 1) * NPAD])
                    wa = wabsp.tile([H, 2 * NPAD], F16, tag="wabs", name="wa")
                    nc.scalar.activation(wa[:], wp[0:H, :], ACTF.Abs)
                    wt = wpool.tile([H, 2 * NPAD], F16, tag="w", name="wt")
                    if b % 2 == 0:
                        nc.vector.tensor_scalar(wt[:], wa[:], 1.0, 1.0,
                                                op0=OP.min, op1=OP.subtract)
                    else:
                        nc.scalar.activation(wt[:], wa[:], ACTF.Relu,
                                             bias=1.0, scale=-1.0)
                    Wh.append(wt)

                for ch in range(3):
                    it = imgp.tile([H, 2 * NPAD], F16, tag="img", name="it")
                    base = (b * 3 + ch) * IMG * IMG
                    eng = nc.gpsimd if ch % 2 == 0 else nc.sync
                    eng.dma_start(
                        out=it[:, 0:464],
                        in_=bass.AP(tensor=xin[:].tensor, offset=base,
                                    ap=[[2 * IMG, H], [1, 464]]))
                    tp = ps_tmp.tile([128, 2 * NPAD], F32, tag="tmp", name="tp2")
                    for xh in range(2):
                        for t in range(2):
                            nc.tensor.matmul(
                                tp[:, xh * NPAD:(xh + 1) * NPAD],
                                it[:, t * IMG + xh * H: t * IMG + xh * H + 128],
                                Wh[t][:, 0:NPAD],
                                start=(t == 0), stop=(t == 1))
                    ts_ = tmpsb.tile([H, 2 * NPAD], F16, tag="tmpsb", name="ts_")
                    if ch % 2 == 0:
                        nc.scalar.copy(ts_[:], tp[0:H, :])
                    else:
                        nc.vector.tensor_copy(ts_[:], tp[0:H, :])
                    cp = ps_crop.tile([128, 2 * NPAD], F32, tag="crop", name="cp")
                    for t in range(2):
                        for xh in range(2):
                            nc.tensor.matmul(
                                cp[:, t * NPAD: t * NPAD + IMG],
                                ts_[:, xh * NPAD + t: xh * NPAD + t + 255: 2],
                                Wh[xh][:, NPAD: NPAD + IMG],
                                start=(xh == 0), stop=(xh == 1))
                    cs = cropsb.tile([H, 2, IMG], F16, tag="cropsb", name="cs")
                    cpv = cp[0:H, :].rearrange("p (t o) -> p t o", t=2)[:, :, 0:IMG]
                    if ch % 2 == 0:
                        nc.vector.tensor_copy(cs[:], cpv)
                    else:
                        nc.scalar.copy(cs[:], cpv)
                    eng2 = nc.sync if ch % 2 == 0 else nc.gpsimd
                    eng2.dma_start(
                        out=crops_o[b, ch].rearrange("(p two) x -> p two x", two=2),
                        in_=cs[:])

        bbox_group(0)
        tail_group(0)
        bbox_group(1)
        tail_group(1)
        interp_group(0)
        interp_group(1)

    if split_waits:
        _split_multi_waits(nc)
    nc.finalize()
    return nc, consts


_CACHE = {}


def _get_program(split_waits=True):
    key = ("prog", split_waits)
    if key not in _CACHE:
        _CACHE[key] = _build_program(split_waits)
    return _CACHE[key]


def _prearrange_attn(attn_slice):
    """(n, 6, 196) f32 -> (84, n, 14) fp16 in (head, j) x (sample, i) layout."""
    n = attn_slice.shape[0]
    a = attn_slice.reshape(n, HEADS, G, G).transpose(1, 2, 0, 3)
    return np.ascontiguousarray(a.reshape(HEADS * G, n, G).astype(np.float16))


def kernel(x, attn_weights):
    x = np.asarray(x)
    attn_weights = np.asarray(attn_weights)
    B = x.shape[0]
    per = B // NC_CORES
    assert per == PB, (B, PB)
    npatch = G * G
    x16 = np.ascontiguousarray(x, dtype=np.float16)
    attn_sl = attn_weights[:, :, 0, -npatch:]  # (B, 6, 196)

    nc, consts = _get_program()
    in_maps = []
    for i in range(NC_CORES):
        m = {k: v for k, v in consts.items()}
        xs = x16[i * per:(i + 1) * per].reshape(-1)
        m["xin"] = np.concatenate([xs, np.zeros(64, np.float16)])
        m["attn"] = _prearrange_attn(attn_sl[i * per:(i + 1) * per])
        in_maps.append(m)
    res = run_bass_kernel_spmd(nc, in_maps, list(range(NC_CORES))).results
    crops = np.concatenate([r["crops"] for r in res], axis=0).astype(np.float32)
    bboxes = np.concatenate([r["bboxes"] for r in res], axis=0).astype(np.int32)
    return crops, bboxes


# revision 50
# speedup vs baseline: 1.0993x; 1.0686x over previous
"""DinoSwav attention-crop kernel for Trainium2 (Bass/Tile), 8-core data parallel.

Per sample:
  1. CLS-attention map (mean over heads) -> 14x14, bilinear-upsampled to 224x224
  2. threshold mask (> 0.6*max), row/col any -> bbox with 10% margin
  3. crop + bilinear resize back to 224x224

Everything runs on device. The crop-resize is expressed as two fp16 matmuls per
channel with data-dependent interpolation matrices Wy/Wx built on-device from
the bbox via a rank-3 outer-difference matmul (int+frac coordinate split keeps
fp16 exact) + min(|d|,1)-1 (negated weights; the sign cancels across the two
matmul stages).
"""

import numpy as np
from contextlib import ExitStack

import concourse.bass as bass
import concourse.tile as tile
from concourse import mybir
from concourse.bass_utils import run_bass_kernel_spmd

F32 = mybir.dt.float32
F16 = mybir.dt.float16
I32 = mybir.dt.int32
AX = mybir.AxisListType
OP = mybir.AluOpType
ACTF = mybir.ActivationFunctionType

IMG = 224
G = 14          # patch grid
HEADS = 6
PB = 16         # samples per core
NC_CORES = 8
H = 112         # half of IMG (partition tile)
NPAD = 256      # padded matmul free dim; two blocks pack into one PSUM bank
BIG = 1 << 20   # sentinel for masked argmin/argmax; exact in f32


def _upsample_matrix():
    """U[x, j]: bilinear 14 -> 224 weights (half-pixel centers). Exact in fp16."""
    s = (np.arange(IMG, dtype=np.float64) + 0.5) * G / IMG - 0.5
    s = np.clip(s, 0.0, G - 1)
    U = np.maximum(0.0, 1.0 - np.abs(s[:, None] - np.arange(G)[None, :]))
    return U.astype(np.float32)  # (224, 14)


def _host_consts():
    U = _upsample_matrix()
    c = {}
    u_rep = np.zeros((HEADS * G, NPAD), np.float32)
    u_rep[:, :IMG] = np.tile(U.T, (HEADS, 1))
    c["u_rep"] = u_rep.astype(np.float16)
    for h in range(2):
        ut = np.zeros((G, 128), np.float32)
        ut[:, :H] = U[h * H:(h + 1) * H, :].T
        c[f"u_t_h{h}"] = ut.astype(np.float16)  # (14, 128): M=128 enables FWL
    u_t_full = np.zeros((G, NPAD), np.float32)
    u_t_full[:, :IMG] = U.T
    c["u_t_full"] = u_t_full.astype(np.float16)
    c["identity112"] = np.eye(H, dtype=np.float32)
    ar = np.arange(IMG, dtype=np.int64)
    blob16 = np.concatenate([
        np.tile((ar - BIG).astype(np.float32), (PB, 1)),
        np.tile((ar + BIG).astype(np.float32), (PB, 1)),
        np.tile((ar + 0.5).astype(np.float32), (PB, 1)),
        np.tile((np.arange(1, 23) * 65536.0).astype(np.float32), (PB, 1)),
    ], axis=1)
    c["blob16"] = blob16  # iota_mB | iota_pB | iota_half_f | thr_pad
    blob3 = []
    for h in range(2):
        w = np.ones((3, 128), np.float32)    # rows: [gi, gf, -x], x = h*112+p
        w[2, :] = -1000.0
        w[2, :H] = -(h * H + np.arange(H, dtype=np.float32))
        blob3.append(w)
    for t in range(2):
        w = np.ones((3, 128), np.float32)    # rows: [gi, gf, -y], y = 2p+t
        w[2, :] = -1000.0
        w[2, :H] = -(2.0 * np.arange(H, dtype=np.float32) + t)
        blob3.append(w)
    c["blob3"] = np.concatenate(blob3, axis=1).astype(np.float16)
    r2i = np.zeros((3, PB * NPAD), np.float32)
    r2i[2, :] = 1.0
    c["r2_init"] = r2i.astype(np.float16)
    return c


def _split_multi_waits(nc, max_waits=1):
    """The walrus build in this environment accepts only one sync-wait per
    instruction; hoist extra waits onto same-engine NOPs placed just before."""
    ctr = 0
    for fn in nc.m.functions:
        for blk in fn.blocks:
            lst = blk.instructions
            out = []
            changed = False
            for ins in lst:
                si = ins.sync_info
                if si is not None and len(si.on_wait) > max_waits:
                    waits = list(si.on_wait)
                    hoist, keep = waits[:-max_waits], waits[-max_waits:]
                    for w in hoist:
                        ctr += 1
                        nop = mybir.InstNoOp(
                            name=f"waitsplit-{ctr}",
                            engine=ins.engine,
                            ins=[], outs=[],
                            sync_info=mybir.SyncInfo(on_wait=[w], on_update=[]),
                        )
                        out.append(nop)
                    si.on_wait = keep
                    changed = True
                out.append(ins)
            if changed:
                blk.instructions = out


def _build_program(split_waits=True):
    nc = bass.Bass()
    xin = nc.declare_dram_parameter("xin", [PB * 3 * IMG * IMG + 64], F16, isOutput=False)
    attn = nc.declare_dram_parameter("attn", [HEADS * G, PB, G], F16, isOutput=False)
    consts = _host_consts()
    cdecl = {}
    for name, arr in consts.items():
        dt = {np.dtype(np.float32): F32, np.dtype(np.float16): F16,
              np.dtype(np.int32): I32}[arr.dtype]
        cdecl[name] = nc.declare_dram_parameter(name, list(arr.shape), dt, isOutput=False)
    crops_o = nc.declare_dram_parameter("crops", [PB, 3, IMG, IMG], F16, isOutput=True)
    bbox_o = nc.declare_dram_parameter("bboxes", [PB, 4], I32, isOutput=True)
    warm_scr = nc.dram_tensor("warm_scr", [H, 2 * NPAD], F16)
    GS = 8  # bbox group size: lets interp(g) overlap bbox(g+1)

    with tile.TileContext(nc) as tc, ExitStack() as ctx:
        singles = ctx.enter_context(tc.tile_pool(name="singles", bufs=1))
        tpairp = ctx.enter_context(tc.tile_pool(name="tpair", bufs=3))
        wpool = ctx.enter_context(tc.tile_pool(name="wpool", bufs=6))
        wabsp = ctx.enter_context(tc.tile_pool(name="wabs", bufs=3))
        imgp = ctx.enter_context(tc.tile_pool(name="imgp", bufs=6))
        tmpsb = ctx.enter_context(tc.tile_pool(name="tmpsb", bufs=6))
        cropsb = ctx.enter_context(tc.tile_pool(name="cropsb", bufs=6))
        smallp = ctx.enter_context(tc.tile_pool(name="smallp", bufs=1))
        ps_mm = ctx.enter_context(tc.tile_pool(name="ps_mm", bufs=3, space="PSUM"))
        ps_tmp = ctx.enter_context(tc.tile_pool(name="ps_tmp", bufs=3, space="PSUM"))
        ps_crop = ctx.enter_context(tc.tile_pool(name="ps_crop", bufs=2, space="PSUM"))

        # ---- load constants into SBUF, round-robin across engine queues so
        #      the ~650ns-per-DMA issue cost is parallel, not serial ----
        csb = {}
        dma_engs = [nc.sync, nc.gpsimd]
        ei = 0
        for name, arr in consts.items():
            if name == "r2_init":
                continue
            t = singles.tile(list(arr.shape), cdecl[name].dtype, tag=name)
            dma_engs[ei % len(dma_engs)].dma_start(out=t[:], in_=cdecl[name][:])
            ei += 1
            csb[name] = t
        for i, nm in enumerate(("iota_mB", "iota_pB", "iota_half_f")):
            csb[nm] = csb["blob16"][:, i * IMG:(i + 1) * IMG]
        csb["thr_pad"] = csb["blob16"][:, 3 * IMG:3 * IMG + 22]
        csb["lhsT_wx0"] = csb["blob3"][:, 0:128]
        csb["lhsT_wx1"] = csb["blob3"][:, 128:256]
        csb["lhsT_wy0"] = csb["blob3"][:, 256:384]
        csb["lhsT_wy1"] = csb["blob3"][:, 384:512]

        lhsT_a = singles.tile([HEADS * G, PB, G], F16, tag="lhsT_a")
        nc.sync.dma_start(out=lhsT_a[:], in_=attn[:])

        # ---- persistent small tensors ----
        Ry = [singles.tile([H, PB], F32, tag=f"Ry{h}", name=f"Ry{h}") for h in range(2)]
        Cxa = singles.tile([H, 2, PB], F32, tag="Cxa", name="Cxa")
        r2 = {a: singles.tile([3, PB * NPAD], F16, tag=f"r2{a}", name=f"r2{a}")
              for a in ("y", "x")}
        nc.sync.dma_start(out=r2["y"][:], in_=cdecl["r2_init"][:])
        nc.gpsimd.dma_start(out=r2["x"][:], in_=cdecl["r2_init"][:])

        # ---- PE warm-up: ~5us of dense matmuls releases the HAM clock gate ----
        wsrc = singles.tile([128, 2 * NPAD], F16, tag="wsrc")
        nc.vector.memset(wsrc[:], 1.0)
        wps = ps_mm.tile([128, 2 * NPAD], F32, tag="mm", name="wps")
        NWARM = 8
        for i in range(NWARM):
            nc.tensor.matmul(wps[:], wsrc[:, 0:128], wsrc[:],
                             start=(i == 0), stop=(i == NWARM - 1))
        wout = singles.tile([H, 2 * NPAD], F16, tag="wout")
        nc.scalar.copy(wout[:], wps[0:H, :])
        nc.sync.dma_start(out=warm_scr[:], in_=wout[:])

        def st(shape, dtype, tag):
            return smallp.tile(shape, dtype, tag=tag, name=tag)

        biasm05 = st([GS, 1], F32, "biasm05")
        nc.vector.memset(biasm05[:], -0.5)

        def bbox_group(g):
            """T' + amap/amapT matmuls + row/col maxes for samples [g*GS, (g+1)*GS)."""
            for b0 in range(g * GS, (g + 1) * GS, 2):
                tpair = tpairp.tile([G, 2 * NPAD], F16, tag="tpair", name="tpair")
                nc.vector.memset(tpair[:, 2 * IMG:2 * NPAD], 0.0)
                for si in range(2):
                    b = b0 + si
                    tp = ps_mm.tile([G, NPAD], F32, tag="mm", name="tp")
                    nc.tensor.matmul(tp[:, 0:IMG], lhsT_a[:, b, :],
                                     csb["u_rep"][:, 0:IMG])
                    nc.scalar.copy(tpair[:, si * IMG:(si + 1) * IMG], tp[:, 0:IMG])
                # upsample-x on partitions: X profile for both samples at once.
                # Along the reduced (y) axis the map is piecewise linear between
                # grid nodes at pixels 16i+7.5, so per-segment pixel maxima are
                # at columns {16i+7, 16i+8}: reducing over those 28 is exact.
                ysel = tpair[:, 0:2 * IMG].rearrange(
                    "p (s a c) -> p s a c", s=2, c=16)[:, :, :, 7:9]
                for h in range(2):
                    am = ps_mm.tile([128, 2 * 28], F32, tag="mm", name="am")
                    nc.tensor.matmul(am[:], csb[f"u_t_h{h}"][:], ysel)
                    nc.vector.tensor_reduce(
                        Ry[h][:, b0:b0 + 2],
                        am[0:H, :].rearrange("p (s c) -> p s c", s=2),
                        axis=AX.X, op=OP.max)
                # upsample-y on free dim: Y profile per sample
                xsel = csb["u_t_full"][:, 0:IMG].rearrange(
                    "p (a c) -> p a c", c=16)[:, :, 7:9]
                for si in range(2):
                    b = b0 + si
                    at = ps_mm.tile([128, 2, 28], F32, tag="mm", name="at")
                    for h in range(2):
                        nc.tensor.matmul(
                            at[:, h, :],
                            tpair[:, si * IMG + h * H: si * IMG + h * H + 128],
                            xsel)
                    nc.vector.tensor_reduce(
                        Cxa[:, :, b], at[0:H, :, :],
                        axis=AX.X, op=OP.max)

        def tail_group(g):
            """Bounds, pads, boxes, interp coords for samples [g*GS, (g+1)*GS)."""
            sl = slice(g * GS, (g + 1) * GS)
            # NOTE: Ry (reduced over T'-free axis) is the X profile, Cx the Y
            # profile: the 14x14 grid reshape puts y on the j (row) index, which
            # lands on the partition axis of T', so the first amap matmul
            # upsamples x on partitions.
            Rt = st([GS, IMG], F32, f"Rt{g}")
            Ct = st([GS, IMG], F32, f"Ct{g}")
            for h in range(2):
                pt = ps_mm.tile([GS, H], F32, tag="mm", name="pt")
                nc.tensor.transpose(pt[:], Ry[h][:, sl], csb["identity112"][:])
                nc.scalar.copy(Rt[:, h * H:(h + 1) * H], pt[:])
                pt2 = ps_mm.tile([GS, H], F32, tag="mm", name="pt2")
                nc.tensor.transpose(pt2[:], Cxa[:, h, sl], csb["identity112"][:])
                nc.scalar.copy(Ct[:, h * H:(h + 1) * H], pt2[:])

            maxv = st([GS, 1], F32, f"maxv{g}")
            nc.vector.tensor_reduce(maxv[:], Rt[:], axis=AX.X, op=OP.max)
            tthr = st([GS, 1], F32, f"tthr{g}")
            nc.scalar.mul(tthr[:], maxv[:], 0.6)
            condf = st([GS, 1], F32, f"condf{g}")
            nc.vector.tensor_scalar(condf[:], maxv[:], 6e-6, None, op0=OP.is_lt)
            notc = st([GS, 1], F32, f"notc{g}")
            nc.vector.tensor_scalar(notc[:], condf[:], -1.0, 1.0,
                                    op0=OP.mult, op1=OP.add)

            # all bound arithmetic in f32 on integer values (exact below 2^21);
            # cast to int32 only at final box assembly.
            fbounds = {}
            box = st([GS, 4], I32, f"box{g}")
            for aname, R in (("x", Rt), ("y", Ct)):
                mf = st([GS, IMG], F32, f"mf{aname}{g}")
                nc.vector.tensor_scalar(mf[:], R[:], tthr[:], None, op0=OP.is_gt)
                cmin = st([GS, IMG], F32, f"cmin{aname}{g}")
                nc.vector.tensor_tensor(cmin[:], mf[:], csb["iota_mB"][0:GS, :],
                                        op=OP.mult)
                nc.vector.tensor_scalar(cmin[:], cmin[:], float(BIG), None, op0=OP.add)
                lo = st([GS, 1], F32, f"lo{aname}{g}")
                nc.vector.tensor_reduce(lo[:], cmin[:], axis=AX.X, op=OP.min)
                cmax = st([GS, IMG], F32, f"cmax{aname}{g}")
                nc.vector.tensor_tensor(cmax[:], mf[:], csb["iota_pB"][0:GS, :],
                                        op=OP.mult)
                nc.vector.tensor_scalar(cmax[:], cmax[:], -float(BIG), None, op0=OP.add)
                hi = st([GS, 1], F32, f"hi{aname}{g}")
                nc.vector.tensor_reduce(hi[:], cmax[:], axis=AX.X, op=OP.max)

                d = st([GS, 1], F32, f"d{aname}{g}")
                nc.vector.tensor_sub(d[:], hi[:], lo[:])
                prodf = st([GS, 1], F32, f"prodf{aname}{g}")
                nc.vector.tensor_scalar(prodf[:], d[:], 6554.0, None, op0=OP.mult)
                mle = st([GS, 22], F32, f"mle{aname}{g}")
                nc.vector.tensor_scalar(mle[:], csb["thr_pad"][0:GS, :], prodf[:],
                                        None, op0=OP.is_le)
                pad = st([GS, 1], F32, f"pad{aname}{g}")
                nc.vector.tensor_reduce(pad[:], mle[:], axis=AX.X, op=OP.add)
                lo1 = st([GS, 1], F32, f"lo1{aname}{g}")
                nc.vector.tensor_sub(lo1[:], lo[:], pad[:])
                nc.vector.tensor_scalar(lo1[:], lo1[:], 0.0, None, op0=OP.max)
                hi1 = st([GS, 1], F32, f"hi1{aname}{g}")
                nc.vector.tensor_add(hi1[:], hi[:], pad[:])
                nc.vector.tensor_scalar(hi1[:], hi1[:], float(IMG), None, op0=OP.min)
                # default box when maxv < 1e-6: lo=0, hi=IMG
                nc.vector.tensor_tensor(lo1[:], lo1[:], notc[:], op=OP.mult)
                nc.vector.tensor_tensor(hi1[:], hi1[:], notc[:], op=OP.mult)
                nc.vector.scalar_tensor_tensor(hi1[:], condf[:], float(IMG), hi1[:],
                                               op0=OP.mult, op1=OP.add)
                # degenerate guard: hi = max(hi, lo+1)
                lop = st([GS, 1], F32, f"lop{aname}{g}")
                nc.vector.tensor_scalar(lop[:], lo1[:], 1.0, None, op0=OP.add)
                nc.vector.tensor_tensor(hi1[:], hi1[:], lop[:], op=OP.max)
                ci = 0 if aname == "x" else 1
                nc.vector.tensor_copy(box[:, ci:ci + 1], lo1[:])
                nc.vector.tensor_copy(box[:, ci + 2:ci + 3], hi1[:])
                fbounds[aname] = (lo1, hi1)
            nc.sync.dma_start(out=bbox_o[sl, :], in_=box[:])

            # interp source coords g = lo + clip(s, 0, n-1), split into an
            # fp16-exact coarse part + small residual (keeps the PE-side
            # outer-difference fp16-accurate)
            for aname in ("y", "x"):
                lo_f, hi_f = fbounds[aname]
                n_f = st([GS, 1], F32, f"nf{aname}{g}")
                nc.vector.tensor_sub(n_f[:], hi_f[:], lo_f[:])
                scale = st([GS, 1], F32, f"scale{aname}{g}")
                nc.vector.tensor_scalar(scale[:], n_f[:], 1.0 / IMG, None, op0=OP.mult)
                upper = st([GS, 1], F32, f"upper{aname}{g}")
                nc.vector.tensor_scalar(upper[:], n_f[:], -1.0, None, op0=OP.add)
                sv = st([GS, IMG], F32, f"sv{aname}{g}")
                nc.scalar.activation(sv[:], csb["iota_half_f"][0:GS, :], ACTF.Relu,
                                     bias=biasm05[:], scale=scale[:])
                gv = st([GS, IMG], F32, f"gv{aname}{g}")
                nc.vector.tensor_scalar(gv[:], sv[:], upper[:], lo_f[:],
                                        op0=OP.min, op1=OP.add)
                ga = st([GS, IMG], F16, f"ga{aname}{g}")
                nc.vector.tensor_copy(ga[:], gv[:])          # coarse (fp16-rounded)
                ga32 = st([GS, IMG], F32, f"ga32{aname}{g}")
                nc.vector.tensor_copy(ga32[:], ga[:])
                gb = st([GS, IMG], F16, f"gb{aname}{g}")
                nc.vector.tensor_sub(gb[:], gv[:], ga32[:])  # residual, |.|<=0.03
                rv = r2[aname][:].rearrange("p (b o) -> p b o", o=NPAD)
                nc.sync.dma_start(out=rv[0:1, sl, 0:IMG], in_=ga[:])
                nc.sync.dma_start(out=rv[1:2, sl, 0:IMG], in_=gb[:])

        def interp_group(g):
            """W build + per-channel interp matmuls for samples [g*GS, (g+1)*GS)."""
            for b in range(g * GS, (g + 1) * GS):
                # W slabs: [ -Wy_t | -Wx_h ] as (112, 512) fp16, negated weights;
                # the sign cancels across the two interp matmul stages.
                Wh = []
                for h in range(2):
                    wp = ps_mm.tile([128, 2 * NPAD], F32, tag="mm", name="wp")
                    nc.tensor.matmul(
                        wp[:, 0:NPAD], csb[f"lhsT_wy{h}"][:],
                        r2["y"][:, b * NPAD:(b + 1) * NPAD])
                    nc.tensor.matmul(
                        wp[:, NPAD:2 * NPAD], csb[f"lhsT_wx{h}"][:],
                        r2["x"][:, b * NPAD:(b + 1) * NPAD])
                    wa = wabsp.tile([H, 2 * NPAD], F16, tag="wabs", name="wa")
                    nc.scalar.activation(wa[:], wp[0:H, :], ACTF.Abs)
                    wt = wpool.tile([H, 2 * NPAD], F16, tag="w", name="wt")
                    # group 0 runs while DVE is walled by the bounds tail ->
                    # finalize on ACT there; group 1 overlaps ACT-heavy interp
                    # -> finalize on DVE. Positive vs negated W both correct
                    # (sign cancels across the two matmul stages).
                    if b < GS:
                        nc.scalar.activation(wt[:], wa[:], ACTF.Relu,
                                             bias=1.0, scale=-1.0)
                    else:
                        nc.vector.tensor_scalar(wt[:], wa[:], 1.0, 1.0,
                                                op0=OP.min, op1=OP.subtract)
                    Wh.append(wt)

                for ch in range(3):
                    it = imgp.tile([H, 2 * NPAD], F16, tag="img", name="it")
                    base = (b * 3 + ch) * IMG * IMG
                    eng = nc.gpsimd if ch % 2 == 0 else nc.sync
                    eng.dma_start(
                        out=it[:, 0:464],
                        in_=bass.AP(tensor=xin[:].tensor, offset=base,
                                    ap=[[2 * IMG, H], [1, 464]]))
                    tp = ps_tmp.tile([128, 2 * NPAD], F32, tag="tmp", name="tp2")
                    for xh in range(2):
                        for t in range(2):
                            nc.tensor.matmul(
                                tp[:, xh * NPAD:(xh + 1) * NPAD],
                                it[:, t * IMG + xh * H: t * IMG + xh * H + 128],
                                Wh[t][:, 0:NPAD],
                                start=(t == 0), stop=(t == 1))
                    ts_ = tmpsb.tile([H, 2 * NPAD], F16, tag="tmpsb", name="ts_")
                    if ch % 2 == 0:
                        nc.scalar.copy(ts_[:], tp[0:H, :])
                    else:
                        nc.vector.tensor_copy(ts_[:], tp[0:H, :])
                    cp = ps_crop.tile([128, 2 * NPAD], F32, tag="crop", name="cp")
                    for t in range(2):
                        for xh in range(2):
                            nc.tensor.matmul(
                                cp[:, t * NPAD: t * NPAD + IMG],
                                ts_[:, xh * NPAD + t: xh * NPAD + t + 255: 2],
                                Wh[xh][:, NPAD: NPAD + IMG],
                                start=(xh == 0), stop=(xh == 1))
                    cs = cropsb.tile([H, 2, IMG], F16, tag="cropsb", name="cs")
                    cpv = cp[0:H, :].rearrange("p (t o) -> p t o", t=2)[:, :, 0:IMG]
                    if ch % 2 == 0:
                        nc.vector.tensor_copy(cs[:], cpv)
                    else:
                        nc.scalar.copy(cs[:], cpv)
                    eng2 = nc.sync if ch % 2 == 0 else nc.gpsimd
                    eng2.dma_start(
                        out=crops_o[b, ch].rearrange("(p two) x -> p two x", two=2),
                        in_=cs[:])

        bbox_group(0)
        tail_group(0)
        bbox_group(1)
        tail_group(1)
        interp_group(0)
        interp_group(1)

    if split_waits:
        _split_multi_waits(nc)
    nc.finalize()
    return nc, consts


_CACHE = {}


def _get_program(split_waits=True):
    key = ("prog", split_waits)
    if key not in _CACHE:
        _CACHE[key] = _build_program(split_waits)
    return _CACHE[key]


def _prearrange_attn(attn_slice):
    """(n, 6, 196) f32 -> (84, n, 14) fp16 in (head, j) x (sample, i) layout."""
    n = attn_slice.shape[0]
    a = attn_slice.reshape(n, HEADS, G, G).transpose(1, 2, 0, 3)
    return np.ascontiguousarray(a.reshape(HEADS * G, n, G).astype(np.float16))


def kernel(x, attn_weights):
    x = np.asarray(x)
    attn_weights = np.asarray(attn_weights)
    B = x.shape[0]
    per = B // NC_CORES
    assert per == PB, (B, PB)
    npatch = G * G
    x16 = np.ascontiguousarray(x, dtype=np.float16)
    attn_sl = attn_weights[:, :, 0, -npatch:]  # (B, 6, 196)

    nc, consts = _get_program()
    in_maps = []
    for i in range(NC_CORES):
        m = {k: v for k, v in consts.items()}
        xs = x16[i * per:(i + 1) * per].reshape(-1)
        m["xin"] = np.concatenate([xs, np.zeros(64, np.float16)])
        m["attn"] = _prearrange_attn(attn_sl[i * per:(i + 1) * per])
        in_maps.append(m)
    res = run_bass_kernel_spmd(nc, in_maps, list(range(NC_CORES))).results
    crops = np.concatenate([r["crops"] for r in res], axis=0).astype(np.float32)
    bboxes = np.concatenate([r["bboxes"] for r in res], axis=0).astype(np.int32)
    return crops, bboxes
